# Initial kernel scaffold
#
"""Trainium2 distributed kernel for nn_Attention (dense transformer attention block).

Strategy (8 NeuronCores, tensor-parallel over heads):
  - Host pre-transposes x_norm -> X^T [C, B*T] (bf16) and slices Wqkv columns
    per core (2 heads/core, deinterleaved RoPE feature order). RoPE sin/cos
    tables precomputed host-side.
  - Each core computes, in bf16 on the TensorEngine:
      1) Q^T/K^T (head-major, D on partitions) + V (natural) for its 2 heads,
         with bias + RoPE fused into the epilogue.
      2) Causal attention, "S^T" flash form without max-subtraction
         (scores ~ N(0,1)): for each K-tile jt and Tq-chunk c:
         S^T[tk, tq] = kT[jt].T @ qT-chunk -> exp (ACT, with 1/sqrt(D) scale,
         triangular mask on the diagonal block) -> P^T tile (SBUF bf16).
         Then two accumulating matmuls per tile: out^T += V[jt].T @ P^T and
         rowsums += ones.T @ P^T (broadcast row-sums on all 128 partitions).
         Normalize with a reciprocal multiply, add V-bias (P rows sum to 1).
      3) Per-batch AllToAll (1 MiB bf16) of out^T row-slices, overlapped with
         the next batch's attention.
      4) Per-batch local out-projection X2 @ Wout (+bout via rank-1 matmul).
  - Host reassembles the per-(core, batch) row pieces -> [B, T, C] fp32.
"""

import numpy as np
import ml_dtypes

import concourse.bass as bass
import concourse.bass_isa as bass_isa
import concourse.mybir as mybir
import concourse.tile as tile
from concourse import bacc
from concourse.bass_utils import run_bass_kernel_spmd
from concourse.masks import make_identity
from concourse.tile_rust import add_dep_helper


N_CORES = 8
B, T, C = 4, 2048, 2048
H, D = 16, 128
ROPE_BASE = 10000.0

BF16 = mybir.dt.bfloat16
F32 = mybir.dt.float32
NPBF16 = ml_dtypes.bfloat16


def _stage1(nc, tc, p, qT_sb, kT_sb, v_sb, wq_sb, wk_sb, wv_sb,
            bq_sb, bk_sb, cos_sb, sin_sb, xT):
    """QKV projection + bias + RoPE into resident SBUF."""
    RC, n_rc, KT, HL, t = p["RC"], p["n_rc"], p["KT"], p["HL"], p["t"]
    dma_engs = [nc.sync, nc.scalar]
    # ---- Q^T / K^T ----
    with (
        tc.tile_pool(name="xin_a", bufs=6) as xin,
        tc.tile_pool(name="ps_a", bufs=2 * 2 * HL, space="PSUM") as psa,
        tc.tile_pool(name="rope", bufs=4) as ropetmp,
    ):
        for rc in range(n_rc):
            r0 = rc * RC
            t0 = r0 % t
            psq = [psa.tile([128, RC], F32, tag="ps_qk", name=f"psq{rc}_{i}")
                   for i in range(2 * HL)]
            for kt in range(KT):
                xt = xin.tile([128, RC], BF16, tag="xt")
                dma_engs[kt % 2].dma_start(out=xt, in_=xT[kt * 128:(kt + 1) * 128, r0:r0 + RC])
                for hm in range(HL):
                    nc.tensor.matmul(psq[hm], lhsT=wq_sb[:, kt, hm * 128:(hm + 1) * 128],
                                     rhs=xt, start=(kt == 0), stop=(kt == KT - 1))
                    nc.tensor.matmul(psq[HL + hm], lhsT=wk_sb[:, kt, hm * 128:(hm + 1) * 128],
                                     rhs=xt, start=(kt == 0), stop=(kt == KT - 1))
            for which, (res, bias_sb) in enumerate(((qT_sb, bq_sb), (kT_sb, bk_sb))):
                for hm in range(HL):
                    dst = res[:, hm, r0:r0 + RC]
                    ps = psq[which * HL + hm]
                    nc.scalar.activation(out=dst, in_=ps,
                                         func=mybir.ActivationFunctionType.Identity,
                                         bias=bias_sb[:, hm:hm + 1], scale=1.0)
                    # RoPE in place: pairs (j, 64+j), angle t*w_j
                    x0 = res[0:64, hm, r0:r0 + RC]
                    x1 = res[64:128, hm, r0:r0 + RC]
                    rt = ropetmp.tile([128, RC], BF16, tag="rt")
                    nc.vector.tensor_mul(rt[0:64, :], x1, sin_sb[64:128, t0:t0 + RC])
                    nc.vector.tensor_mul(rt[64:128, :], x0, sin_sb[0:64, t0:t0 + RC])
                    nc.vector.tensor_mul(x0, x0, cos_sb[0:64, t0:t0 + RC])
                    nc.vector.tensor_sub(x0, x0, rt[0:64, :])
                    nc.vector.tensor_mul(x1, x1, cos_sb[64:128, t0:t0 + RC])
                    nc.vector.tensor_add(x1, x1, rt[64:128, :])
    # ---- V (natural layout) ----
    with (
        tc.tile_pool(name="xin_b", bufs=6) as xin,
        tc.tile_pool(name="ps_b", bufs=2 * (RC // 128), space="PSUM") as psb,
    ):
        for rc in range(n_rc):
            r0 = rc * RC
            psv = [psb.tile([128, p["HD"]], F32, tag="ps_v", name=f"psv{rc}_{i}")
                   for i in range(RC // 128)]
            for kt in range(KT):
                xt = xin.tile([128, RC], BF16, tag="xt")
                dma_engs[kt % 2].dma_start(out=xt, in_=xT[kt * 128:(kt + 1) * 128, r0:r0 + RC])
                for rs_ in range(RC // 128):
                    nc.tensor.matmul(psv[rs_], lhsT=xt[:, rs_ * 128:(rs_ + 1) * 128],
                                     rhs=wv_sb[:, kt, :], start=(kt == 0), stop=(kt == KT - 1))
            for rs_ in range(RC // 128):
                rt_ = (r0 // 128) + rs_
                nc.scalar.activation(out=v_sb[:, rt_, :], in_=psv[rs_],
                                     func=mybir.ActivationFunctionType.Copy, scale=1.0)


def _attn_head(nc, p, pools, bb, hm, qT_sb, kT_sb, v_sb, bv_sb, maskU_sb,
               ones_f32, a2a_in_b):
    """S^T-form causal attention for one (batch, local head) -> a2a_in_b."""
    t, HL, HD, d = p["t"], p["HL"], p["HD"], p["d"]
    SCALE = p["SCALE"]
    NCH = t // 512                      # Tq chunks
    seg = t // N_CORES                  # rows per a2a slot
    att, attsm, rcpp, spsum, opsum, oTpool = pools
    qT_h = qT_sb[:, hm, bb * t:(bb + 1) * t]
    kT_h = kT_sb[:, hm, bb * t:(bb + 1) * t]
    oT = oTpool.tile([128, t], BF16, tag="oT", name=f"oT{bb}_{hm}")
    markers = []
    for c in range(NCH):
        tq0 = c * 512
        jt_max = 4 * (c + 1)
        psum_o = opsum.tile([128, 512], F32, tag="po", name=f"po{bb}_{hm}_{c}")
        rs_d = rcpp.tile([128, 512], F32, tag="rsd", name=f"rsd{bb}_{hm}_{c}")

        def rs_accum(jt, pt_ap, off):
            if jt == 0:
                nc.vector.tensor_copy(rs_d, pt_ap)
            else:
                nc.vector.tensor_add(rs_d[:, off:512], rs_d[:, off:512], pt_ap[:, off:512])

        # non-diagonal K-tile pairs: one fat exp per pair
        for jp in range(2 * c):
            jt0 = 2 * jp
            ps2 = spsum.tile([128, 1024], F32, tag="s", name=f"st{bb}_{hm}_{c}_{jp}")
            nc.tensor.matmul(ps2[:, 0:512], lhsT=kT_h[:, jt0 * 128:(jt0 + 1) * 128],
                             rhs=qT_h[:, tq0:tq0 + 512], start=True, stop=True)
            nc.tensor.matmul(ps2[:, 512:1024], lhsT=kT_h[:, (jt0 + 1) * 128:(jt0 + 2) * 128],
                             rhs=qT_h[:, tq0:tq0 + 512], start=True, stop=True)
            pT2 = att.tile([128, 1024], BF16, tag="pT", name=f"pT{bb}_{hm}_{c}_{jp}")
            nc.scalar.activation(out=pT2, in_=ps2,
                                 func=mybir.ActivationFunctionType.Exp, scale=SCALE)
            for u in range(2):
                jt = jt0 + u
                pv_mm = nc.tensor.matmul(
                    psum_o,
                    lhsT=v_sb[:, (bb * t) // 128 + jt, hm * d:(hm + 1) * d],
                    rhs=pT2[:, u * 512:(u + 1) * 512],
                    start=(jt == 0), stop=False)
                rs_accum(jt, pT2[:, u * 512:(u + 1) * 512].rearrange("p n -> p n"), 0)
        # diagonal K-tiles (off > 0 or triangular mask)
        for jt in range(4 * c, jt_max):
            off = jt * 128 - tq0
            ps2 = spsum.tile([128, 1024], F32, tag="s", name=f"std{bb}_{hm}_{c}_{jt}")
            ps_st = ps2[:, 0:512]
            nc.tensor.matmul(ps_st[:, off:512],
                             lhsT=kT_h[:, jt * 128:(jt + 1) * 128],
                             rhs=qT_h[:, tq0 + off:tq0 + 512],
                             start=True, stop=True)
            pT2 = att.tile([128, 1024], BF16, tag="pT", name=f"pTd{bb}_{hm}_{c}_{jt}")
            pT = pT2[:, 0:512]
            tmp_d = attsm.tile([128, 128], BF16, tag="tmpd")
            nc.scalar.activation(out=tmp_d, in_=ps_st[:, off:off + 128],
                                 func=mybir.ActivationFunctionType.Exp, scale=SCALE)
            nc.vector.tensor_mul(pT[:, off:off + 128], tmp_d, maskU_sb)
            if off + 128 < 512:
                nc.scalar.activation(out=pT[:, off + 128:512],
                                     in_=ps_st[:, off + 128:512],
                                     func=mybir.ActivationFunctionType.Exp, scale=SCALE)
            pv_mm = nc.tensor.matmul(
                psum_o[:, off:512],
                lhsT=v_sb[:, (bb * t) // 128 + jt, hm * d:(hm + 1) * d],
                rhs=pT[:, off:512],
                start=(jt == 0), stop=(jt == jt_max - 1))
            rs_accum(jt, pT, off)
        markers.append(pv_mm)
        # row-sums: cross-partition reduce on GpSimd, fast reciprocal, normalize
        rs_red = rcpp.tile([128, 512], F32, tag="rsr", name=f"rsr{bb}_{hm}_{c}")
        nc.gpsimd.partition_all_reduce(rs_red, rs_d, 128, bass_isa.ReduceOp.add)
        rcp = rcpp.tile([128, 512], F32, tag="rcp")
        nc.vector.reciprocal_approx_fast(out=rcp, in_=rs_red)
        o_sb = rcpp.tile([128, 512], F32, tag="osb", name=f"osb{bb}_{hm}_{c}")
        nc.scalar.activation(out=o_sb, in_=psum_o,
                             func=mybir.ActivationFunctionType.Copy, scale=1.0)
        nc.vector.tensor_mul(oT[:, tq0:tq0 + 512], o_sb, rcp)
        nc.vector.tensor_scalar_add(oT[:, tq0:tq0 + 512], oT[:, tq0:tq0 + 512],
                                    bv_sb[:, hm:hm + 1])
    for sl in range(N_CORES):
        nc.gpsimd.dma_start(
            out=a2a_in_b[sl * HD + hm * d: sl * HD + (hm + 1) * d, :],
            in_=oT[:, sl * seg:(sl + 1) * seg])
    return markers


def _outproj_piece(nc, p, pools, bb, nns, a2a_out_b, wo, bo_sb, ones1, out,
                   after=None):
    """Out-projection piece (some outcol chunks) for this core's row-piece of bb."""
    _outproj_piece._gidx = 0
    c, KT = p["c"], p["KT"]
    seg = p["t"] // N_CORES             # rows in this piece
    x2pool, wop, p3pool, o3pool = pools
    if nns[0] == 0:
        x2t = x2pool.tile([128, KT, seg], BF16, tag="x2t", name=f"x2t{bb}")
        nc.sync.dma_start(out=x2t, in_=a2a_out_b[:, :].rearrange("(kt p) r -> p kt r", p=128))
        _outproj_piece._x2t[bb] = x2t
    x2t = _outproj_piece._x2t[bb]
    for nn_ in nns:
        wo_tiles = []
        for kt in range(KT):
            wt = wop.tile([128, 512], BF16, tag="wo", name=f"wo{bb}_{nn_}_{kt}")
            eng = nc.gpsimd if kt % 2 else nc.sync
            eng.dma_start(out=wt, in_=wo[kt * 128:(kt + 1) * 128,
                                         nn_ * 512:(nn_ + 1) * 512])
            wo_tiles.append(wt)
        MT = min(128, seg)
        for m in range(seg // MT):
            ps3 = p3pool.tile([MT, 512], F32, tag="ps3", name=f"ps3{bb}_{nn_}_{m}")
            for kt in range(KT):
                mm = nc.tensor.matmul(ps3, lhsT=x2t[:, kt, m * MT:(m + 1) * MT],
                                      rhs=wo_tiles[kt], start=(kt == 0), stop=False)
                if kt == 0 and after:
                    g = _outproj_piece._gidx
                    mk = after[min(1 + g, len(after) - 1)]
                    add_dep_helper(mm.ins, mk.ins, sync=False,
                                   reason="stagger outproj behind attention chunks")
                    _outproj_piece._gidx += 1
            nc.tensor.matmul(ps3[0:MT, :], lhsT=ones1[0:1, 0:MT],
                             rhs=bo_sb[0:1, nn_ * 512:(nn_ + 1) * 512],
                             start=False, stop=True)
            o3 = o3pool.tile([MT, 512], F32, tag="o3", name=f"o3{bb}_{nn_}_{m}")
            nc.scalar.activation(out=o3, in_=ps3,
                                 func=mybir.ActivationFunctionType.Copy, scale=1.0)
            nc.sync.dma_start(out=out[bb * seg + m * MT:bb * seg + (m + 1) * MT,
                                      nn_ * 512:(nn_ + 1) * 512], in_=o3)


_outproj_piece._x2t = {}
_outproj_piece._gidx = 0


def build_nc(b=B, t=T, c=C, h=H, d=D, n_cores=N_CORES, stages=3):
    HL = h // n_cores          # heads per core
    R = b * t                  # total rows
    RS = R // n_cores          # rows per core overall
    RC = 512                   # row-chunk for stage 1
    p = dict(b=b, t=t, c=c, h=h, d=d, HL=HL, R=R, RS=RS, RC=RC,
             n_rc=R // RC, KT=c // 128, NQT=t // 128, HD=HL * d,
             SCALE=1.0 / float(np.sqrt(d)))
    seg = t // n_cores

    nc = bacc.Bacc(None, target_bir_lowering=False, debug=False,
                   num_devices=n_cores)

    xT = nc.declare_dram_parameter("xT", [c, R], BF16, isOutput=False)
    wq = nc.declare_dram_parameter("wq", [c, p["HD"]], BF16, isOutput=False)
    wk = nc.declare_dram_parameter("wk", [c, p["HD"]], BF16, isOutput=False)
    wv = nc.declare_dram_parameter("wv", [c, p["HD"]], BF16, isOutput=False)
    bq = nc.declare_dram_parameter("bq", [128, HL], F32, isOutput=False)
    bk = nc.declare_dram_parameter("bk", [128, HL], F32, isOutput=False)
    bv = nc.declare_dram_parameter("bv", [128, HL], F32, isOutput=False)
    wo = nc.declare_dram_parameter("wo", [c, c], BF16, isOutput=False)
    bo = nc.declare_dram_parameter("bo", [1, c], BF16, isOutput=False)
    cosT = nc.declare_dram_parameter("cosT", [128, t], BF16, isOutput=False)
    sinT = nc.declare_dram_parameter("sinT", [128, t], BF16, isOutput=False)
    maskc = nc.declare_dram_parameter("maskc", [128, 128], BF16, isOutput=False)
    out = nc.declare_dram_parameter("out", [RS, c], F32, isOutput=True)

    with tile.TileContext(nc) as tc:
        with (
            tc.tile_pool(name="consts", bufs=1) as consts,
            tc.tile_pool(name="qkvres", bufs=1) as qkvres,
            tc.tile_pool(name="dram", bufs=1, space="DRAM") as dram,
        ):
            # ---- constants into SBUF ----
            wq_sb = consts.tile([128, p["KT"], p["HD"]], BF16, tag="wq")
            wk_sb = consts.tile([128, p["KT"], p["HD"]], BF16, tag="wk")
            wv_sb = consts.tile([128, p["KT"], p["HD"]], BF16, tag="wv")
            nc.sync.dma_start(out=wq_sb, in_=wq[:, :].rearrange("(kt p) n -> p kt n", p=128))
            nc.sync.dma_start(out=wk_sb, in_=wk[:, :].rearrange("(kt p) n -> p kt n", p=128))
            nc.sync.dma_start(out=wv_sb, in_=wv[:, :].rearrange("(kt p) n -> p kt n", p=128))
            bq_sb = consts.tile([128, HL], F32, tag="bq")
            bk_sb = consts.tile([128, HL], F32, tag="bk")
            bv_sb = consts.tile([128, HL], F32, tag="bv")
            nc.sync.dma_start(out=bq_sb, in_=bq[:, :])
            nc.sync.dma_start(out=bk_sb, in_=bk[:, :])
            nc.sync.dma_start(out=bv_sb, in_=bv[:, :])
            cos_sb = consts.tile([128, t], BF16, tag="cos")
            sin_sb = consts.tile([128, t], BF16, tag="sin")
            nc.sync.dma_start(out=cos_sb, in_=cosT[:, :])
            nc.sync.dma_start(out=sin_sb, in_=sinT[:, :])
            maskU_sb = consts.tile([128, 128], BF16, tag="mask")
            nc.sync.dma_start(out=maskU_sb, in_=maskc[:, :])
            bo_sb = consts.tile([1, c], BF16, tag="bo")
            nc.sync.dma_start(out=bo_sb, in_=bo[:, :])
            ones1 = consts.tile([1, 128], BF16, tag="ones1")
            nc.vector.memset(ones1, 1.0)
            ones_f32 = consts.tile([128, 128], F32, tag="ones128")
            nc.vector.memset(ones_f32, 1.0)

            # ---- resident QKV (bf16) ----
            qT_sb = qkvres.tile([128, HL, p["R"]], BF16, tag="qT")   # [d, h, row]
            kT_sb = qkvres.tile([128, HL, p["R"]], BF16, tag="kT")
            v_sb = qkvres.tile([128, p["R"] // 128, p["HD"]], BF16, tag="v")

            _stage1(nc, tc, p, qT_sb, kT_sb, v_sb, wq_sb, wk_sb, wv_sb,
                    bq_sb, bk_sb, cos_sb, sin_sb, xT)

            a2a_ins = []
            a2a_outs = []
            for bb in range(b):
                a2a_ins.append(dram.tile([n_cores * p["HD"], seg], BF16,
                                         tag=f"a2a_in{bb}", name=f"a2a_in{bb}"))
                a2a_outs.append(dram.tile([n_cores * p["HD"], seg], BF16,
                                          tag=f"a2a_out{bb}", name=f"a2a_out{bb}"))

            with (
                tc.tile_pool(name="att", bufs=3) as att,
                tc.tile_pool(name="attsm", bufs=4) as attsm,
                tc.tile_pool(name="rcpp", bufs=2) as rcpp,
                tc.tile_pool(name="spsum", bufs=2, space="PSUM") as spsum,
                tc.tile_pool(name="opsum", bufs=2, space="PSUM") as opsum,
                tc.tile_pool(name="oTp", bufs=2) as oTpool,
                tc.tile_pool(name="x2", bufs=1) as x2pool,
                tc.tile_pool(name="wop", bufs=24) as wop,
                tc.tile_pool(name="p3", bufs=2, space="PSUM") as p3pool,
                tc.tile_pool(name="o3", bufs=3) as o3pool,
            ):
                apools = (att, attsm, rcpp, spsum, opsum, oTpool)
                opools = (x2pool, wop, p3pool, o3pool)
                _outproj_piece._x2t = {}
                nnh = (c // 512) // 2   # nn chunks per piece
                for bb in range(b):
                    for hm in range(HL):
                        mk = _attn_head(nc, p, apools, bb, hm, qT_sb, kT_sb, v_sb,
                                        bv_sb, maskU_sb, ones_f32, a2a_ins[bb])
                        if stages >= 3 and bb > 0 and hm == 0:
                            _outproj_piece(nc, p, opools, bb - 1,
                                           list(range(nnh)),
                                           a2a_outs[bb - 1], wo, bo_sb, ones1, out,
                                           after=mk)
                    if stages >= 3:
                        nc.gpsimd.collective_compute(
                            "AllToAll", mybir.AluOpType.bypass,
                            replica_groups=[list(range(n_cores))],
                            ins=[a2a_ins[bb][:, :].opt()],
                            outs=[a2a_outs[bb][:, :].opt()],
                        )
                        if bb > 0:
                            _outproj_piece(nc, p, opools, bb - 1,
                                           list(range(nnh, c // 512)),
                                           a2a_outs[bb - 1], wo, bo_sb, ones1, out,
                                           after=mk)
                if stages >= 3:
                    _outproj_piece(nc, p, opools, b - 1, list(range(nnh)),
                                   a2a_outs[b - 1], wo, bo_sb, ones1, out,
                                   after=mk)
                    _outproj_piece(nc, p, opools, b - 1, list(range(nnh, c // 512)),
                                   a2a_outs[b - 1], wo, bo_sb, ones1, out,
                                   after=mk)

    nc.compile()
    return nc


def _host_prep(x_norm, Wqkv, bqkv, Wout, bout, b, t, c, h, d, n_cores):
    """Build per-core input maps (numpy, bf16)."""
    HL = h // n_cores
    R = b * t
    perm = np.concatenate([np.arange(0, d, 2), np.arange(1, d, 2)])  # deinterleave

    XT = np.ascontiguousarray(x_norm.reshape(R, c).T.astype(NPBF16))
    inv_freq = 1.0 / (ROPE_BASE ** (np.arange(0, d, 2, dtype=np.float64) / d))
    ang = np.arange(t, dtype=np.float64)[None, :] * inv_freq[:, None]  # [d/2, t]
    cosT = np.concatenate([np.cos(ang), np.cos(ang)], axis=0).astype(NPBF16)
    sinT = np.concatenate([np.sin(ang), np.sin(ang)], axis=0).astype(NPBF16)
    # upper-triangular (incl diagonal) mask for the transposed P layout
    maskc = np.triu(np.ones((128, 128), dtype=np.float32)).astype(NPBF16)
    wo_b = np.ascontiguousarray(Wout.astype(NPBF16))
    bo_b = bout.reshape(1, c).astype(NPBF16)

    in_maps = []
    for i in range(n_cores):
        cols_q = np.concatenate([i * HL * d + hh * d + perm for hh in range(HL)])
        cols_k = cols_q + h * d
        cols_v = np.concatenate([2 * h * d + i * HL * d + hh * d + np.arange(d)
                                 for hh in range(HL)])
        wq_i = np.ascontiguousarray(Wqkv[:, cols_q].astype(NPBF16))
        wk_i = np.ascontiguousarray(Wqkv[:, cols_k].astype(NPBF16))
        wv_i = np.ascontiguousarray(Wqkv[:, cols_v].astype(NPBF16))
        bq_i = np.stack([bqkv[i * HL * d + hh * d + perm] for hh in range(HL)],
                        axis=1).astype(np.float32)
        bk_i = np.stack([bqkv[h * d + i * HL * d + hh * d + perm] for hh in range(HL)],
                        axis=1).astype(np.float32)
        bv_i = np.stack([bqkv[2 * h * d + i * HL * d + hh * d + np.arange(d)]
                         for hh in range(HL)], axis=1).astype(np.float32)
        in_maps.append({
            "xT": XT, "wq": wq_i, "wk": wk_i, "wv": wv_i,
            "bq": np.ascontiguousarray(bq_i), "bk": np.ascontiguousarray(bk_i),
            "bv": np.ascontiguousarray(bv_i),
            "wo": wo_b, "bo": bo_b, "cosT": cosT, "sinT": sinT, "maskc": maskc,
        })
    return in_maps


def _gather(parts, b, t, c, n_cores):
    """Core j's out rows are, for each batch bb, global rows
    [bb*t + j*seg, bb*t + (j+1)*seg) with seg = t // n_cores."""
    seg = t // n_cores
    R = b * t
    full = np.empty((R, c), dtype=np.float32)
    for j in range(n_cores):
        for bb in range(b):
            full[bb * t + j * seg: bb * t + (j + 1) * seg] = \
                parts[j][bb * seg:(bb + 1) * seg]
    return full.reshape(b, t, c)


_NC_CACHE = {}


def kernel(x_norm, Wqkv, bqkv, Wout, bout):
    b, t, c = x_norm.shape
    h = 16
    d = c // h
    key = (b, t, c)
    if key not in _NC_CACHE:
        _NC_CACHE[key] = build_nc(b, t, c, h, d, N_CORES)
    nc = _NC_CACHE[key]
    in_maps = _host_prep(np.asarray(x_norm, dtype=np.float32),
                         np.asarray(Wqkv, dtype=np.float32),
                         np.asarray(bqkv, dtype=np.float32),
                         np.asarray(Wout, dtype=np.float32),
                         np.asarray(bout, dtype=np.float32),
                         b, t, c, h, d, N_CORES)
    res = run_bass_kernel_spmd(nc, in_maps, core_ids=list(range(N_CORES)))
    parts = [np.asarray(res.results[i]["out"], dtype=np.float32) for i in range(N_CORES)]
    return _gather(parts, b, t, c, N_CORES)



# revision 1
# speedup vs baseline: 1.1108x; 1.1108x over previous
"""Trainium2 distributed kernel for nn_Attention (dense transformer attention block).

Strategy (8 NeuronCores, tensor-parallel over heads):
  - Host pre-transposes x_norm -> X^T [C, B*T] (bf16) and slices Wqkv columns
    per core (2 heads/core, deinterleaved RoPE feature order). RoPE sin/cos
    tables precomputed host-side.
  - Each core computes, in bf16 on the TensorEngine:
      1) Q^T/K^T (head-major, D on partitions) + V (natural) for its 2 heads,
         with bias + RoPE fused into the epilogue.
      2) Causal attention, "S^T" flash form without max-subtraction
         (scores ~ N(0,1)): for each K-tile jt and Tq-chunk c:
         S^T[tk, tq] = kT[jt].T @ qT-chunk -> exp (ACT, with 1/sqrt(D) scale,
         triangular mask on the diagonal block) -> P^T tile (SBUF bf16).
         Then two accumulating matmuls per tile: out^T += V[jt].T @ P^T and
         rowsums += ones.T @ P^T (broadcast row-sums on all 128 partitions).
         Normalize with a reciprocal multiply, add V-bias (P rows sum to 1).
      3) Per-batch AllToAll (1 MiB bf16) of out^T row-slices, overlapped with
         the next batch's attention.
      4) Per-batch local out-projection X2 @ Wout (+bout via rank-1 matmul).
  - Host reassembles the per-(core, batch) row pieces -> [B, T, C] fp32.
"""

import numpy as np
import ml_dtypes

import concourse.bass as bass
import concourse.bass_isa as bass_isa
import concourse.mybir as mybir
import concourse.tile as tile
from concourse import bacc
from concourse.bass_utils import run_bass_kernel_spmd
from concourse.masks import make_identity
from concourse.tile_rust import add_dep_helper


N_CORES = 8
B, T, C = 4, 2048, 2048
H, D = 16, 128
ROPE_BASE = 10000.0

BF16 = mybir.dt.bfloat16
F32 = mybir.dt.float32
NPBF16 = ml_dtypes.bfloat16


def _stage1(nc, tc, p, qT_sb, kT_sb, v_sb, wq_sb, wk_sb, wv_sb,
            bq_sb, bk_sb, cos_sb, sin_sb, xT):
    """QKV projection + bias + RoPE into resident SBUF."""
    RC, n_rc, KT, HL, t = p["RC"], p["n_rc"], p["KT"], p["HL"], p["t"]
    dma_engs = [nc.sync, nc.scalar]
    # ---- Q^T / K^T ----
    with (
        tc.tile_pool(name="xin_a", bufs=6) as xin,
        tc.tile_pool(name="ps_a", bufs=2 * 2 * HL, space="PSUM") as psa,
        tc.tile_pool(name="rope", bufs=4) as ropetmp,
    ):
        for rc in range(n_rc):
            r0 = rc * RC
            t0 = r0 % t
            psq = [psa.tile([128, RC], F32, tag="ps_qk", name=f"psq{rc}_{i}")
                   for i in range(2 * HL)]
            for kt in range(KT):
                xt = xin.tile([128, RC], BF16, tag="xt")
                dma_engs[kt % 2].dma_start(out=xt, in_=xT[kt * 128:(kt + 1) * 128, r0:r0 + RC])
                for hm in range(HL):
                    nc.tensor.matmul(psq[hm], lhsT=wq_sb[:, kt, hm * 128:(hm + 1) * 128],
                                     rhs=xt, start=(kt == 0), stop=(kt == KT - 1))
                    nc.tensor.matmul(psq[HL + hm], lhsT=wk_sb[:, kt, hm * 128:(hm + 1) * 128],
                                     rhs=xt, start=(kt == 0), stop=(kt == KT - 1))
            for which, (res, bias_sb) in enumerate(((qT_sb, bq_sb), (kT_sb, bk_sb))):
                for hm in range(HL):
                    dst = res[:, hm, r0:r0 + RC]
                    ps = psq[which * HL + hm]
                    nc.scalar.activation(out=dst, in_=ps,
                                         func=mybir.ActivationFunctionType.Identity,
                                         bias=bias_sb[:, hm:hm + 1], scale=1.0)
                    # RoPE in place: pairs (j, 64+j), angle t*w_j
                    x0 = res[0:64, hm, r0:r0 + RC]
                    x1 = res[64:128, hm, r0:r0 + RC]
                    rt = ropetmp.tile([128, RC], BF16, tag="rt")
                    nc.vector.tensor_mul(rt[0:64, :], x1, sin_sb[64:128, t0:t0 + RC])
                    nc.vector.tensor_mul(rt[64:128, :], x0, sin_sb[0:64, t0:t0 + RC])
                    nc.vector.tensor_mul(x0, x0, cos_sb[0:64, t0:t0 + RC])
                    nc.vector.tensor_sub(x0, x0, rt[0:64, :])
                    nc.vector.tensor_mul(x1, x1, cos_sb[64:128, t0:t0 + RC])
                    nc.vector.tensor_add(x1, x1, rt[64:128, :])
    # ---- V (natural layout) ----
    with (
        tc.tile_pool(name="xin_b", bufs=6) as xin,
        tc.tile_pool(name="ps_b", bufs=2 * (RC // 128), space="PSUM") as psb,
    ):
        for rc in range(n_rc):
            r0 = rc * RC
            psv = [psb.tile([128, p["HD"]], F32, tag="ps_v", name=f"psv{rc}_{i}")
                   for i in range(RC // 128)]
            for kt in range(KT):
                xt = xin.tile([128, RC], BF16, tag="xt")
                dma_engs[kt % 2].dma_start(out=xt, in_=xT[kt * 128:(kt + 1) * 128, r0:r0 + RC])
                for rs_ in range(RC // 128):
                    nc.tensor.matmul(psv[rs_], lhsT=xt[:, rs_ * 128:(rs_ + 1) * 128],
                                     rhs=wv_sb[:, kt, :], start=(kt == 0), stop=(kt == KT - 1))
            for rs_ in range(RC // 128):
                rt_ = (r0 // 128) + rs_
                nc.scalar.activation(out=v_sb[:, rt_, :], in_=psv[rs_],
                                     func=mybir.ActivationFunctionType.Copy, scale=1.0)


def _attn_head(nc, p, pools, bb, hm, qT_sb, kT_sb, v_sb, bv_sb, maskU_sb,
               ones_f32, a2a_in_b):
    """S^T-form causal attention for one (batch, local head) -> a2a_in_b."""
    t, HL, HD, d = p["t"], p["HL"], p["HD"], p["d"]
    SCALE = p["SCALE"]
    NCH = t // 512                      # Tq chunks
    seg = t // N_CORES                  # rows per a2a slot
    att, attsm, rcpp, spsum, opsum, oTpool = pools
    qT_h = qT_sb[:, hm, bb * t:(bb + 1) * t]
    kT_h = kT_sb[:, hm, bb * t:(bb + 1) * t]
    oT = oTpool.tile([128, t], BF16, tag="oT", name=f"oT{bb}_{hm}")
    markers = []
    for c in range(NCH):
        tq0 = c * 512
        jt_max = 4 * (c + 1)
        psum_o = opsum.tile([128, 512], F32, tag="po", name=f"po{bb}_{hm}_{c}")
        rs_d = rcpp.tile([128, 512], F32, tag="rsd", name=f"rsd{bb}_{hm}_{c}")

        def rs_accum(jt, pt_ap, off):
            if jt == 0:
                nc.vector.tensor_copy(rs_d, pt_ap)
            else:
                nc.vector.tensor_add(rs_d[:, off:512], rs_d[:, off:512], pt_ap[:, off:512])

        # non-diagonal K-tile pairs: one fat exp per pair
        for jp in range(2 * c):
            jt0 = 2 * jp
            ps2 = spsum.tile([128, 1024], F32, tag="s", name=f"st{bb}_{hm}_{c}_{jp}")
            nc.tensor.matmul(ps2[:, 0:512], lhsT=kT_h[:, jt0 * 128:(jt0 + 1) * 128],
                             rhs=qT_h[:, tq0:tq0 + 512], start=True, stop=True)
            nc.tensor.matmul(ps2[:, 512:1024], lhsT=kT_h[:, (jt0 + 1) * 128:(jt0 + 2) * 128],
                             rhs=qT_h[:, tq0:tq0 + 512], start=True, stop=True)
            pT2 = att.tile([128, 1024], BF16, tag="pT", name=f"pT{bb}_{hm}_{c}_{jp}")
            nc.scalar.activation(out=pT2, in_=ps2,
                                 func=mybir.ActivationFunctionType.Exp, scale=SCALE)
            for u in range(2):
                jt = jt0 + u
                pv_mm = nc.tensor.matmul(
                    psum_o,
                    lhsT=v_sb[:, (bb * t) // 128 + jt, hm * d:(hm + 1) * d],
                    rhs=pT2[:, u * 512:(u + 1) * 512],
                    start=(jt == 0), stop=False)
                rs_accum(jt, pT2[:, u * 512:(u + 1) * 512].rearrange("p n -> p n"), 0)
        # diagonal K-tiles (off > 0 or triangular mask)
        for jt in range(4 * c, jt_max):
            off = jt * 128 - tq0
            ps2 = spsum.tile([128, 1024], F32, tag="s", name=f"std{bb}_{hm}_{c}_{jt}")
            ps_st = ps2[:, 0:512]
            nc.tensor.matmul(ps_st[:, off:512],
                             lhsT=kT_h[:, jt * 128:(jt + 1) * 128],
                             rhs=qT_h[:, tq0 + off:tq0 + 512],
                             start=True, stop=True)
            pT2 = att.tile([128, 1024], BF16, tag="pT", name=f"pTd{bb}_{hm}_{c}_{jt}")
            pT = pT2[:, 0:512]
            tmp_d = attsm.tile([128, 128], BF16, tag="tmpd")
            nc.scalar.activation(out=tmp_d, in_=ps_st[:, off:off + 128],
                                 func=mybir.ActivationFunctionType.Exp, scale=SCALE)
            nc.vector.tensor_mul(pT[:, off:off + 128], tmp_d, maskU_sb)
            if off + 128 < 512:
                nc.scalar.activation(out=pT[:, off + 128:512],
                                     in_=ps_st[:, off + 128:512],
                                     func=mybir.ActivationFunctionType.Exp, scale=SCALE)
            pv_mm = nc.tensor.matmul(
                psum_o[:, off:512],
                lhsT=v_sb[:, (bb * t) // 128 + jt, hm * d:(hm + 1) * d],
                rhs=pT[:, off:512],
                start=(jt == 0), stop=(jt == jt_max - 1))
            rs_accum(jt, pT, off)
        markers.append(pv_mm)
        # row-sums: cross-partition reduce on GpSimd, fast reciprocal, normalize
        rs_red = rcpp.tile([128, 512], F32, tag="rsr", name=f"rsr{bb}_{hm}_{c}")
        nc.gpsimd.partition_all_reduce(rs_red, rs_d, 128, bass_isa.ReduceOp.add)
        rcp = rcpp.tile([128, 512], F32, tag="rcp")
        nc.vector.reciprocal_approx_fast(out=rcp, in_=rs_red)
        o_sb = rcpp.tile([128, 512], F32, tag="osb", name=f"osb{bb}_{hm}_{c}")
        nc.scalar.activation(out=o_sb, in_=psum_o,
                             func=mybir.ActivationFunctionType.Copy, scale=1.0)
        nc.vector.tensor_mul(oT[:, tq0:tq0 + 512], o_sb, rcp)
        nc.vector.tensor_scalar_add(oT[:, tq0:tq0 + 512], oT[:, tq0:tq0 + 512],
                                    bv_sb[:, hm:hm + 1])
    for sl in range(N_CORES):
        nc.gpsimd.dma_start(
            out=a2a_in_b[sl * HD + hm * d: sl * HD + (hm + 1) * d, :],
            in_=oT[:, sl * seg:(sl + 1) * seg])
    return markers


def _outproj_piece(nc, p, pools, bb, nns, a2a_out_b, wo, bo_sb, ones1, out,
                   after=None):
    """Out-projection piece (some outcol chunks) for this core's row-piece of bb."""
    _outproj_piece._gidx = 0
    c, KT = p["c"], p["KT"]
    seg = p["t"] // N_CORES             # rows in this piece
    x2pool, wop, p3pool, o3pool = pools
    if nns[0] == 0:
        x2t = x2pool.tile([128, KT, seg], BF16, tag="x2t", name=f"x2t{bb}")
        nc.sync.dma_start(out=x2t, in_=a2a_out_b[:, :].rearrange("(kt p) r -> p kt r", p=128))
        _outproj_piece._x2t[bb] = x2t
    x2t = _outproj_piece._x2t[bb]
    for nn_ in nns:
        wo_tiles = []
        for kt in range(KT):
            wt = wop.tile([128, 512], BF16, tag="wo", name=f"wo{bb}_{nn_}_{kt}")
            eng = nc.gpsimd if kt % 2 else nc.sync
            eng.dma_start(out=wt, in_=wo[kt * 128:(kt + 1) * 128,
                                         nn_ * 512:(nn_ + 1) * 512])
            wo_tiles.append(wt)
        MT = min(128, seg)
        for m in range(seg // MT):
            ps3 = p3pool.tile([MT, 512], F32, tag="ps3", name=f"ps3{bb}_{nn_}_{m}")
            for kt in range(KT):
                mm = nc.tensor.matmul(ps3, lhsT=x2t[:, kt, m * MT:(m + 1) * MT],
                                      rhs=wo_tiles[kt], start=(kt == 0), stop=False)
                if kt == 0 and after:
                    g = _outproj_piece._gidx
                    mk = after[min(1 + g, len(after) - 1)]
                    add_dep_helper(mm.ins, mk.ins, sync=False,
                                   reason="stagger outproj behind attention chunks")
                    _outproj_piece._gidx += 1
            nc.tensor.matmul(ps3[0:MT, :], lhsT=ones1[0:1, 0:MT],
                             rhs=bo_sb[0:1, nn_ * 512:(nn_ + 1) * 512],
                             start=False, stop=True)
            o3 = o3pool.tile([MT, 512], F32, tag="o3", name=f"o3{bb}_{nn_}_{m}")
            nc.scalar.activation(out=o3, in_=ps3,
                                 func=mybir.ActivationFunctionType.Copy, scale=1.0)
            nc.sync.dma_start(out=out[bb * seg + m * MT:bb * seg + (m + 1) * MT,
                                      nn_ * 512:(nn_ + 1) * 512], in_=o3)


_outproj_piece._x2t = {}
_outproj_piece._gidx = 0


def build_nc(b=B, t=T, c=C, h=H, d=D, n_cores=N_CORES, stages=3):
    HL = h // n_cores          # heads per core
    R = b * t                  # total rows
    RS = R // n_cores          # rows per core overall
    RC = 512                   # row-chunk for stage 1
    p = dict(b=b, t=t, c=c, h=h, d=d, HL=HL, R=R, RS=RS, RC=RC,
             n_rc=R // RC, KT=c // 128, NQT=t // 128, HD=HL * d,
             SCALE=1.0 / float(np.sqrt(d)))
    seg = t // n_cores

    nc = bacc.Bacc(None, target_bir_lowering=False, debug=False,
                   num_devices=n_cores)

    xT = nc.declare_dram_parameter("xT", [c, R], BF16, isOutput=False)
    wq = nc.declare_dram_parameter("wq", [c, p["HD"]], BF16, isOutput=False)
    wk = nc.declare_dram_parameter("wk", [c, p["HD"]], BF16, isOutput=False)
    wv = nc.declare_dram_parameter("wv", [c, p["HD"]], BF16, isOutput=False)
    bq = nc.declare_dram_parameter("bq", [128, HL], F32, isOutput=False)
    bk = nc.declare_dram_parameter("bk", [128, HL], F32, isOutput=False)
    bv = nc.declare_dram_parameter("bv", [128, HL], F32, isOutput=False)
    wo = nc.declare_dram_parameter("wo", [c, c], BF16, isOutput=False)
    bo = nc.declare_dram_parameter("bo", [1, c], BF16, isOutput=False)
    cosT = nc.declare_dram_parameter("cosT", [128, t], BF16, isOutput=False)
    sinT = nc.declare_dram_parameter("sinT", [128, t], BF16, isOutput=False)
    maskc = nc.declare_dram_parameter("maskc", [128, 128], BF16, isOutput=False)
    out = nc.declare_dram_parameter("out", [RS, c], F32, isOutput=True)

    with tile.TileContext(nc) as tc:
        with (
            tc.tile_pool(name="consts", bufs=1) as consts,
            tc.tile_pool(name="qkvres", bufs=1) as qkvres,
            tc.tile_pool(name="dram", bufs=1, space="DRAM") as dram,
        ):
            # ---- constants into SBUF ----
            wq_sb = consts.tile([128, p["KT"], p["HD"]], BF16, tag="wq")
            wk_sb = consts.tile([128, p["KT"], p["HD"]], BF16, tag="wk")
            wv_sb = consts.tile([128, p["KT"], p["HD"]], BF16, tag="wv")
            nc.sync.dma_start(out=wq_sb, in_=wq[:, :].rearrange("(kt p) n -> p kt n", p=128))
            nc.sync.dma_start(out=wk_sb, in_=wk[:, :].rearrange("(kt p) n -> p kt n", p=128))
            nc.sync.dma_start(out=wv_sb, in_=wv[:, :].rearrange("(kt p) n -> p kt n", p=128))
            bq_sb = consts.tile([128, HL], F32, tag="bq")
            bk_sb = consts.tile([128, HL], F32, tag="bk")
            bv_sb = consts.tile([128, HL], F32, tag="bv")
            nc.sync.dma_start(out=bq_sb, in_=bq[:, :])
            nc.sync.dma_start(out=bk_sb, in_=bk[:, :])
            nc.sync.dma_start(out=bv_sb, in_=bv[:, :])
            cos_sb = consts.tile([128, t], BF16, tag="cos")
            sin_sb = consts.tile([128, t], BF16, tag="sin")
            nc.sync.dma_start(out=cos_sb, in_=cosT[:, :])
            nc.sync.dma_start(out=sin_sb, in_=sinT[:, :])
            maskU_sb = consts.tile([128, 128], BF16, tag="mask")
            nc.sync.dma_start(out=maskU_sb, in_=maskc[:, :])
            bo_sb = consts.tile([1, c], BF16, tag="bo")
            nc.sync.dma_start(out=bo_sb, in_=bo[:, :])
            ones1 = consts.tile([1, 128], BF16, tag="ones1")
            nc.vector.memset(ones1, 1.0)
            ones_f32 = consts.tile([128, 128], F32, tag="ones128")
            nc.vector.memset(ones_f32, 1.0)

            # ---- resident QKV (bf16) ----
            qT_sb = qkvres.tile([128, HL, p["R"]], BF16, tag="qT")   # [d, h, row]
            kT_sb = qkvres.tile([128, HL, p["R"]], BF16, tag="kT")
            v_sb = qkvres.tile([128, p["R"] // 128, p["HD"]], BF16, tag="v")

            _stage1(nc, tc, p, qT_sb, kT_sb, v_sb, wq_sb, wk_sb, wv_sb,
                    bq_sb, bk_sb, cos_sb, sin_sb, xT)

            a2a_ins = []
            a2a_outs = []
            for bb in range(b):
                a2a_ins.append(dram.tile([n_cores * p["HD"], seg], BF16,
                                         tag=f"a2a_in{bb}", name=f"a2a_in{bb}"))
                a2a_outs.append(dram.tile([n_cores * p["HD"], seg], BF16,
                                          tag=f"a2a_out{bb}", name=f"a2a_out{bb}"))

            with (
                tc.tile_pool(name="att", bufs=3) as att,
                tc.tile_pool(name="attsm", bufs=4) as attsm,
                tc.tile_pool(name="rcpp", bufs=2) as rcpp,
                tc.tile_pool(name="spsum", bufs=2, space="PSUM") as spsum,
                tc.tile_pool(name="opsum", bufs=2, space="PSUM") as opsum,
                tc.tile_pool(name="oTp", bufs=2) as oTpool,
                tc.tile_pool(name="x2", bufs=1) as x2pool,
                tc.tile_pool(name="wop", bufs=24) as wop,
                tc.tile_pool(name="p3", bufs=2, space="PSUM") as p3pool,
                tc.tile_pool(name="o3", bufs=3) as o3pool,
            ):
                apools = (att, attsm, rcpp, spsum, opsum, oTpool)
                opools = (x2pool, wop, p3pool, o3pool)
                _outproj_piece._x2t = {}
                nnh = (c // 512) // 2   # nn chunks per piece
                for bb in range(b):
                    for hm in range(HL):
                        mk = _attn_head(nc, p, apools, bb, hm, qT_sb, kT_sb, v_sb,
                                        bv_sb, maskU_sb, ones_f32, a2a_ins[bb])
                        if stages >= 3 and bb > 0 and hm == 0:
                            _outproj_piece(nc, p, opools, bb - 1,
                                           list(range(nnh)),
                                           a2a_outs[bb - 1], wo, bo_sb, ones1, out,
                                           after=mk)
                    if stages >= 3:
                        nc.gpsimd.collective_compute(
                            "AllToAll", mybir.AluOpType.bypass,
                            replica_groups=[list(range(n_cores))],
                            ins=[a2a_ins[bb][:, :].opt()],
                            outs=[a2a_outs[bb][:, :].opt()],
                        )
                        if bb > 0:
                            _outproj_piece(nc, p, opools, bb - 1,
                                           list(range(nnh, c // 512)),
                                           a2a_outs[bb - 1], wo, bo_sb, ones1, out,
                                           after=mk)
                if stages >= 3:
                    _outproj_piece(nc, p, opools, b - 1, list(range(nnh)),
                                   a2a_outs[b - 1], wo, bo_sb, ones1, out,
                                   after=mk)
                    _outproj_piece(nc, p, opools, b - 1, list(range(nnh, c // 512)),
                                   a2a_outs[b - 1], wo, bo_sb, ones1, out,
                                   after=mk)

    nc.compile()
    return nc


def _host_prep(x_norm, Wqkv, bqkv, Wout, bout, b, t, c, h, d, n_cores):
    """Build per-core input maps (numpy, bf16)."""
    HL = h // n_cores
    R = b * t
    perm = np.concatenate([np.arange(0, d, 2), np.arange(1, d, 2)])  # deinterleave

    XT = np.ascontiguousarray(x_norm.reshape(R, c).T.astype(NPBF16))
    inv_freq = 1.0 / (ROPE_BASE ** (np.arange(0, d, 2, dtype=np.float64) / d))
    ang = np.arange(t, dtype=np.float64)[None, :] * inv_freq[:, None]  # [d/2, t]
    cosT = np.concatenate([np.cos(ang), np.cos(ang)], axis=0).astype(NPBF16)
    sinT = np.concatenate([np.sin(ang), np.sin(ang)], axis=0).astype(NPBF16)
    # upper-triangular (incl diagonal) mask for the transposed P layout
    maskc = np.triu(np.ones((128, 128), dtype=np.float32)).astype(NPBF16)
    wo_b = np.ascontiguousarray(Wout.astype(NPBF16))
    bo_b = bout.reshape(1, c).astype(NPBF16)

    in_maps = []
    for i in range(n_cores):
        cols_q = np.concatenate([i * HL * d + hh * d + perm for hh in range(HL)])
        cols_k = cols_q + h * d
        cols_v = np.concatenate([2 * h * d + i * HL * d + hh * d + np.arange(d)
                                 for hh in range(HL)])
        wq_i = np.ascontiguousarray(Wqkv[:, cols_q].astype(NPBF16))
        wk_i = np.ascontiguousarray(Wqkv[:, cols_k].astype(NPBF16))
        wv_i = np.ascontiguousarray(Wqkv[:, cols_v].astype(NPBF16))
        bq_i = np.stack([bqkv[i * HL * d + hh * d + perm] for hh in range(HL)],
                        axis=1).astype(np.float32)
        bk_i = np.stack([bqkv[h * d + i * HL * d + hh * d + perm] for hh in range(HL)],
                        axis=1).astype(np.float32)
        bv_i = np.stack([bqkv[2 * h * d + i * HL * d + hh * d + np.arange(d)]
                         for hh in range(HL)], axis=1).astype(np.float32)
        in_maps.append({
            "xT": XT, "wq": wq_i, "wk": wk_i, "wv": wv_i,
            "bq": np.ascontiguousarray(bq_i), "bk": np.ascontiguousarray(bk_i),
            "bv": np.ascontiguousarray(bv_i),
            "wo": wo_b, "bo": bo_b, "cosT": cosT, "sinT": sinT, "maskc": maskc,
        })
    return in_maps


def _gather(parts, b, t, c, n_cores):
    """Core j's out rows are, for each batch bb, global rows
    [bb*t + j*seg, bb*t + (j+1)*seg) with seg = t // n_cores."""
    seg = t // n_cores
    R = b * t
    full = np.empty((R, c), dtype=np.float32)
    for j in range(n_cores):
        for bb in range(b):
            full[bb * t + j * seg: bb * t + (j + 1) * seg] = \
                parts[j][bb * seg:(bb + 1) * seg]
    return full.reshape(b, t, c)


_NC_CACHE = {}


def kernel(x_norm, Wqkv, bqkv, Wout, bout):
    b, t, c = x_norm.shape
    h = 16
    d = c // h
    key = (b, t, c)
    if key not in _NC_CACHE:
        _NC_CACHE[key] = build_nc(b, t, c, h, d, N_CORES)
    nc = _NC_CACHE[key]
    in_maps = _host_prep(np.asarray(x_norm, dtype=np.float32),
                         np.asarray(Wqkv, dtype=np.float32),
                         np.asarray(bqkv, dtype=np.float32),
                         np.asarray(Wout, dtype=np.float32),
                         np.asarray(bout, dtype=np.float32),
                         b, t, c, h, d, N_CORES)
    res = run_bass_kernel_spmd(nc, in_maps, core_ids=list(range(N_CORES)))
    parts = [np.asarray(res.results[i]["out"], dtype=np.float32) for i in range(N_CORES)]
    return _gather(parts, b, t, c, N_CORES)

